# revision 46
# baseline (speedup 1.0000x reference)
"""Log-sparse attention kernel for 8 TRN2 NeuronCores.

Sharding: batch (4) x head-group (2 groups of 4 heads) = 8 cores, no
collectives.  Per core:
  - causal-conv QK projection as tap-paired K=128 bf16 matmuls (host ships a
    one-shifted duplicate of x^T so two taps share one matmul),
  - transposed scores ST[kj,qi] = K^T Q in bf16 with exact causal widths,
    exp on ScalarE straight out of PSUM (1/sqrt(64) folded into the
    activation scale), multiplicative log-sparse mask in bf16 on VectorE,
  - attention*V with V pre-multiplied by the output projection on the host
    (Wv @ Wp) plus a ones-column, so one matmul chain yields both the
    projected partial output (transposed) and the softmax denominators,
  - attention weights emitted via PE transpose + PSUM copyback + DMA as
    unnormalized bf16; the host applies the 1/rowsum normalization and
    upcast while unsharding.
Emission is software-pipelined across head pairs with descending block
indices so the next pair's score tiles reuse pmt pool slots freed
diagonally by the current pair's output phase.  Only the causal lower
triangle is computed; the PJRT path zero-initializes output buffers so the
upper triangle is never written.
"""

from contextlib import ExitStack

import numpy as np
import ml_dtypes

import concourse.bacc as bacc
import concourse.tile as tile
import concourse.mybir as mybir

B, T, E, H, QL, SUB = 4, 2048, 64, 8, 6, 64
HG = 4          # heads per core
NCORES = 8
NT = T // 128   # 16 blocks of 128 along t/qi/kj

F32 = mybir.dt.float32
F32R = mybir.dt.float32r
BF16 = mybir.dt.bfloat16
BNP = ml_dtypes.bfloat16

def _qi0(kb):
    """First qi column computed for kj-block kb (exact causal start)."""
    return kb * 128


def _class(w):
    for c in (512, 1024, 1536, 2048):
        if w <= c:
            return c
    return 2048


_MT_OFF = []
_o = 0
for _kb in range(NT):
    _MT_OFF.append(_o)
    _o += T - _qi0(_kb)
MT_W = _o  # 20480


def _build_nc():
    nc = bacc.Bacc("TRN2", target_bir_lowering=False, debug=False,
                   num_devices=NCORES)
    xT = nc.dram_tensor("xT", [E + 1, T + QL - 1], BF16, kind="ExternalInput").ap()
    xT2 = nc.dram_tensor("xT2", [128, T + QL - 1], BF16, kind="ExternalInput").ap()
    wqk = nc.dram_tensor("wqk", [128, QL // 2, 2 * HG * E], BF16, kind="ExternalInput").ap()
    cbias = nc.dram_tensor("cbias", [128, 4], F32, kind="ExternalInput").ap()
    wv = nc.dram_tensor("wv", [E + 1, HG * (E + 1)], BF16, kind="ExternalInput").ap()
    mt = nc.dram_tensor("mt", [128, MT_W], BF16, kind="ExternalInput").ap()
    ident = nc.dram_tensor("ident", [128, 128], BF16, kind="ExternalInput").ap()
    w_out = nc.dram_tensor("w", [HG, T, T], BF16, kind="ExternalOutput").ap()
    po_out = nc.dram_tensor("po", [HG, E, T], BF16, kind="ExternalOutput").ap()
    rv_out = nc.dram_tensor("rv", [HG, T], F32, kind="ExternalOutput").ap()

    Exp = mybir.ActivationFunctionType.Exp

    with tile.TileContext(nc) as tc, ExitStack() as ctx:
        const = ctx.enter_context(tc.tile_pool(name="const", bufs=1))
        qk_pool = ctx.enter_context(tc.tile_pool(name="qkT", bufs=1))
        v_pool = ctx.enter_context(tc.tile_pool(name="vall", bufs=1))
        w_pool = ctx.enter_context(tc.tile_pool(name="wrow", bufs=5))
        pmt_pools = {
            kb: ctx.enter_context(
                tc.tile_pool(name=f"pmt{kb}", bufs=(3 if kb >= 6 else 2)))
            for kb in range(NT)
        }
        r_pool = ctx.enter_context(tc.tile_pool(name="rinv", bufs=4))
        po_pool = ctx.enter_context(tc.tile_pool(name="posb", bufs=1))

        # ---- load inputs -------------------------------------------------
        xT_sb = const.tile([E + 1, T + QL - 1], BF16)
        nc.sync.dma_start(xT_sb[:], xT[:])
        xT2_sb = const.tile([128, T + QL - 1], BF16)
        nc.sync.dma_start(xT2_sb[:], xT2[:])
        wqk_sb = const.tile([128, QL // 2, 2 * HG * E], BF16)
        nc.sync.dma_start(wqk_sb[:], wqk[:])
        cbias_sb = const.tile([128, 4], F32)
        nc.sync.dma_start(cbias_sb[:], cbias[:])
        wv_sb = const.tile([E + 1, HG * (E + 1)], BF16)
        nc.sync.dma_start(wv_sb[:], wv[:])
        ident_sb = const.tile([128, 128], BF16)
        nc.sync.dma_start(ident_sb[:], ident[:])
        mt_sb = const.tile([128, MT_W], BF16)
        nc.sync.dma_start(mt_sb[:], mt[:])

        # ---- main psum pools (shared with conv/value via tag) -----------
        main_ctx = ExitStack()
        ps_st = main_ctx.enter_context(
            tc.tile_pool(name="psst", bufs=3, space="PSUM"))
        ps_u = main_ctx.enter_context(
            tc.tile_pool(name="psu", bufs=2, space="PSUM"))
        ps_tr = main_ctx.enter_context(
            tc.tile_pool(name="pstr", bufs=3, space="PSUM"))

        ones_sb = const.tile([1, 1], BF16)
        nc.vector.memset(ones_sb[:], 1.0)
        poT_sb = po_pool.tile([E, HG, T], BF16)

        # qkT in two tiles so heads 0/1 (ct 0,2) never depend on the
        # conv writes for heads 2/3 (ct 1,3).
        qkT02 = qk_pool.tile([128, 2, T], BF16, name="qkT02")
        qkT13 = qk_pool.tile([128, 2, T], BF16, name="qkT13")
        v_sb = v_pool.tile([128, NT, HG * (E + 1)], BF16)

        def emit_conv(ct, tt):
            # 6-tap causal conv for one (channel-tile, t-tile): qk channels
            dst = qkT02 if ct in (0, 2) else qkT13
            di = 0 if ct < 2 else 1
            cps = ps_st.tile([128, 512], F32, name="cps", tag="st")
            for p in range(QL // 2):
                nc.tensor.matmul(
                    cps[:, :],
                    lhsT=wqk_sb[:, p, ct * 128:(ct + 1) * 128],
                    rhs=xT2_sb[:, tt * 512 + 2 * p: tt * 512 + 2 * p + 512],
                    start=(p == 0), stop=(p == QL // 2 - 1),
                )
            nc.vector.tensor_scalar_add(
                dst[:, di, tt * 512:(tt + 1) * 512], cps[:, :],
                cbias_sb[:, ct:ct + 1])

        def emit_value(tb):
            vps = ps_st.tile([128, HG * (E + 1)], F32, name="vps", tag="st")
            nc.tensor.matmul(
                vps[:, :],
                lhsT=xT_sb[:, QL - 1 + tb * 128: QL - 1 + (tb + 1) * 128],
                rhs=wv_sb[:],
                start=True, stop=True,
            )
            nc.vector.tensor_copy(v_sb[:, tb, :], vps[:, :])

        pmt_map = [[None] * NT for _ in range(HG)]

        def emit_A2(pair, kb):
            # Paired heads (2*pair, 2*pair+1) live at partition offsets 0/64:
            # their K=64 score matmuls go to distinct PE row-groups and run
            # concurrently.  ST = K^T Q, exp (1/8 folded), log-sparse mask.
            qk_t = qkT02 if pair == 0 else qkT13
            q0 = _qi0(kb)
            wdt = T - q0
            pa = pmt_pools[kb].tile([128, wdt], BF16, name="pmtA", tag="pmt")
            pb = pmt_pools[kb].tile([128, wdt], BF16, name="pmtB", tag="pmt")
            for p0, pt in ((0, pa), (64, pb)):
                for seg in range(q0, T, 512):
                    nn = min(512, T - seg)
                    st_t = ps_st.tile([128, 512], F32, name="st_t", tag="st")
                    nc.tensor.matmul(
                        st_t[:, 0:nn],
                        lhsT=qk_t[p0:p0 + 64, 1, kb * 128:(kb + 1) * 128],
                        rhs=qk_t[p0:p0 + 64, 0, seg: seg + nn],
                        start=True, stop=True)
                    nc.scalar.activation(
                        pt[:, seg - q0: seg - q0 + nn], st_t[:, 0:nn],
                        Exp, scale=0.125)
            nc.vector.tensor_mul(
                pa[:, :], pa[:, :], mt_sb[:, _MT_OFF[kb]: _MT_OFF[kb] + wdt])
            nc.vector.tensor_mul(
                pb[:, :], pb[:, :], mt_sb[:, _MT_OFF[kb]: _MT_OFF[kb] + wdt])
            pmt_map[2 * pair][kb] = pa
            pmt_map[2 * pair + 1][kb] = pb

        def emit_B(hh, qt):
            # U = V^T P (unnormalized attn out, transposed); ones-column row
            # gives softmax denominators, transposed into [qi,1] via tiny
            # matmuls against a [1,1] ones tile.
            u_t = ps_u.tile([E + 1, 512], F32)
            for kb in range(4 * qt + 4):
                dd = max(0, kb * 128 - qt * 512)
                src0 = qt * 512 + dd - _qi0(kb)
                nc.tensor.matmul(
                    u_t[:, dd:512],
                    lhsT=v_sb[:, kb, hh * (E + 1):(hh + 1) * (E + 1)],
                    rhs=pmt_map[hh][kb][:, src0: src0 + 512 - dd],
                    start=(kb == 0), stop=(kb == 4 * qt + 3))
            nc.vector.tensor_copy(
                poT_sb[:, hh, qt * 512:(qt + 1) * 512], u_t[0:E, :])
            r_chunk = r_pool.tile([1, 512], F32, tag="rrow")
            nc.vector.tensor_copy(r_chunk[0:1, :], u_t[E:E + 1, :])
            nc.sync.dma_start(
                rv_out[hh, qt * 512:(qt + 1) * 512], r_chunk[0:1, :])

        nchunk = [0]

        def emit_C(hh, qb):
            # attn-weight output row qb: PE-transpose back to [qi,kj], copy
            # out of PSUM, DMA the causal prefix (unnormalized bf16; the
            # host applies the softmax denominators).
            wrow = w_pool.tile([128, T], BF16)
            for c0 in range(0, (qb + 1) * 128, 1024):
                cw = min(1024, (qb + 1) * 128 - c0)
                tr = ps_tr.tile([128, 1024], BF16)
                for j in range(0, cw, 128):
                    kbj = (c0 + j) // 128
                    nc.tensor.transpose(
                        tr[:, j:j + 128],
                        pmt_map[hh][kbj][:, qb * 128 - _qi0(kbj):
                                         qb * 128 - _qi0(kbj) + 128],
                        ident_sb[:])
                if nchunk[0] % 3 == 0:
                    nc.scalar.copy(wrow[:, c0:c0 + cw], tr[:, 0:cw])
                else:
                    nc.vector.tensor_copy(wrow[:, c0:c0 + cw], tr[:, 0:cw])
                nchunk[0] += 1
            nc.sync.dma_start(
                w_out[hh, qb * 128:(qb + 1) * 128, 0:(qb + 1) * 128],
                wrow[:, 0:(qb + 1) * 128])

        # Software pipeline, ascending blocks: with exact causal widths,
        # B(qt) needs only score tiles kb <= 4qt+3 and C(qb) needs kb <= qb,
        # so each head-pair self-overlaps its score (ACT-heavy), AV (PE) and
        # weight-output (PE+DVE) phases, with C trailing A/B by one group.
        prequeue = []
        for tt in range(4):
            prequeue += [("c", 0, tt), ("c", 2, tt), ("v", tt, 0)]
        rest = ([("v", tb, 0) for tb in range(4, NT)]
                + [("c", 1, tt) for tt in range(4)]
                + [("c", 3, tt) for tt in range(4)])
        def emit_item(item):
            if item[0] == "c":
                emit_conv(item[1], item[2])
            else:
                emit_value(item[1])
        for item in prequeue:
            emit_item(item)
        ri = 0
        for pair in range(2):
            ha, hb = 2 * pair, 2 * pair + 1
            for g in range(4):
                for kb in range(4 * g, 4 * g + 4):
                    emit_A2(pair, kb)
                    if pair == 0 and ri < len(rest):
                        emit_item(rest[ri]); ri += 1
                emit_B(ha, g)
                emit_B(hb, g)
                if g >= 1:
                    for qb in range(4 * (g - 1), 4 * g):
                        emit_C(ha, qb)
                        emit_C(hb, qb)
            if pair == 0:
                while ri < len(rest):
                    emit_item(rest[ri]); ri += 1
            for qb in range(12, 16):
                emit_C(ha, qb)
                emit_C(hb, qb)
        main_ctx.close()

        nc.sync.dma_start(po_out.rearrange("h e t -> e h t"), poT_sb[:])

    nc.compile()
    return nc


# --------------------------------------------------------------------------
# Host-side sharding / unsharding and the cached PJRT runner.
# --------------------------------------------------------------------------

_RUNNER = None


class _Runner:
    def __init__(self):
        import jax
        from jax.experimental.shard_map import shard_map
        from jax.sharding import Mesh, PartitionSpec
        from concourse.bass2jax import (
            _bass_exec_p, install_neuronx_cc_hook, partition_id_tensor)

        install_neuronx_cc_hook()
        nc = self.nc = _build_nc()
        partition_name = (
            nc.partition_id_tensor.name if nc.partition_id_tensor else None)

        in_names, out_names, out_avals, zero_shapes = [], [], [], []
        for alloc in nc.m.functions[0].allocations:
            if not isinstance(alloc, mybir.MemoryLocationSet):
                continue
            name = alloc.memorylocations[0].name
            if alloc.kind == "ExternalInput":
                if name != partition_name:
                    in_names.append(name)
            elif alloc.kind == "ExternalOutput":
                out_names.append(name)
                shape = tuple(alloc.tensor_shape)
                dtype = mybir.dt.np(alloc.dtype)
                out_avals.append(jax.core.ShapedArray(shape, dtype))
                zero_shapes.append((shape, dtype))
        self.in_names = in_names
        self.out_names = out_names
        self.out_avals = out_avals
        self.zero_shapes = zero_shapes
        n_params = len(in_names)
        n_outs = len(out_names)
        all_names = in_names + out_names
        if partition_name is not None:
            all_names = all_names + [partition_name]

        def _body(*args):
            operands = list(args)
            if partition_name is not None:
                operands.append(partition_id_tensor())
            outs = _bass_exec_p.bind(
                *operands,
                out_avals=tuple(out_avals),
                in_names=tuple(all_names),
                out_names=tuple(out_names),
                lowering_input_output_aliases=(),
                sim_require_finite=True,
                sim_require_nnan=True,
                nc=nc,
            )
            return tuple(outs)

        devices = jax.devices()[:NCORES]
        mesh = Mesh(np.asarray(devices), ("core",))
        in_specs = (PartitionSpec("core"),) * (n_params + n_outs)
        out_specs = (PartitionSpec("core"),) * n_outs
        donate = tuple(range(n_params, n_params + n_outs))
        self.sharded = jax.jit(
            shard_map(_body, mesh=mesh, in_specs=in_specs,
                      out_specs=out_specs, check_rep=False),
            donate_argnums=donate, keep_unused=True,
        )

    def execute(self, in_maps):
        concat_in = [
            np.concatenate([np.asarray(m[name]) for m in in_maps], axis=0)
            for name in self.in_names
        ]
        concat_zeros = [
            np.zeros((NCORES * s[0], *s[1:]), d) for (s, d) in self.zero_shapes
        ]
        out_arrs = self.sharded(*concat_in, *concat_zeros)
        return [
            {
                name: np.asarray(out_arrs[i]).reshape(
                    NCORES, *self.out_avals[i].shape)[c]
                for i, name in enumerate(self.out_names)
            }
            for c in range(NCORES)
        ]


def _get_runner():
    global _RUNNER
    if _RUNNER is None:
        _RUNNER = _Runner()
    return _RUNNER


def _prep_in_maps(x, Wqk, bqk, Wv, bv, Wp, bp, mask):
    x = np.asarray(x, np.float32)
    Wqk = np.asarray(Wqk, np.float32)
    bqk = np.asarray(bqk, np.float32)
    Wv = np.asarray(Wv, np.float32)
    bv = np.asarray(bv, np.float32)
    Wp = np.asarray(Wp, np.float32)
    m = np.asarray(mask).reshape(T, T).astype(np.float32)

    # mask, transposed + causally packed (shared by all cores)
    mt_np = np.zeros((128, MT_W), dtype=BNP)
    for kb in range(NT):
        q0 = _qi0(kb)
        blk = m[q0:, kb * 128:(kb + 1) * 128].T  # [128, T-q0]
        mt_np[:, _MT_OFF[kb]: _MT_OFF[kb] + T - q0] = blk.astype(BNP)
    ident_np = np.eye(128, dtype=BNP)

    in_maps = []
    for core in range(NCORES):
        b, g = divmod(core, 2)
        heads = [4 * g + i for i in range(HG)]

        xT_np = np.zeros((E + 1, T + QL - 1), np.float32)
        xT_np[:E, QL - 1:] = x[b].T
        xT_np[E, :] = 1.0

        # channel order: q of the 4 heads (64 each), then k of the 4 heads
        chan = np.concatenate(
            [np.arange(64 * h, 64 * h + 64) for h in heads]
            + [np.arange(512 + 64 * h, 512 + 64 * h + 64) for h in heads])
        # tap-paired conv weights: rows 0:64 = tap 2p, rows 64:128 = tap 2p+1
        wqk_np = np.zeros((128, QL // 2, 2 * HG * E), np.float32)
        wt = Wqk[chan].transpose(1, 2, 0)  # [e, dt, ci]
        for p in range(QL // 2):
            wqk_np[:E, p] = wt[:, 2 * p]
            wqk_np[E:, p] = wt[:, 2 * p + 1]
        cbias_np = bqk[chan].reshape(4, 128).T.astype(np.float32).copy()
        xT2_np = np.zeros((128, T + QL - 1), np.float32)
        xT2_np[:E] = xT_np[:E]
        xT2_np[E:, :-1] = xT_np[:E, 1:]

        wv_np = np.zeros((E + 1, HG * (E + 1)), np.float32)
        for i, h in enumerate(heads):
            wvp = Wv[:, 64 * h: 64 * h + 64] @ Wp[64 * h: 64 * h + 64, :]
            wv_np[:E, i * (E + 1): i * (E + 1) + E] = wvp
            wv_np[E, i * (E + 1): i * (E + 1) + E] = (
                bv[64 * h: 64 * h + 64] @ Wp[64 * h: 64 * h + 64, :])
            wv_np[E, i * (E + 1) + E] = 1.0

        in_maps.append({
            "xT": xT_np.astype(BNP),
            "wqk": wqk_np.astype(BNP),
            "xT2": xT2_np.astype(BNP),
            "cbias": cbias_np,
            "wv": wv_np.astype(BNP),
            "mt": mt_np,
            "ident": ident_np,
        })
    return in_maps


def _assemble(results, bp):
    bp = np.asarray(bp, np.float32)
    attn = np.empty((B, H, T, T), np.float32)
    out = np.empty((B, T, E), np.float32)
    for core in range(NCORES):
        b, g = divmod(core, 2)
        rinv = 1.0 / np.asarray(results[core]["rv"], np.float32)  # [HG, T]
        w = results[core]["w"]                                    # bf16 [HG,T,T]
        for hh in range(HG):
            np.multiply(np.asarray(w[hh], np.float32),
                        rinv[hh][:, None], out=attn[b, 4 * g + hh])
    for b in range(B):
        acc = np.zeros((T, E), np.float32)
        for core in (2 * b, 2 * b + 1):
            poT = np.asarray(results[core]["po"], np.float32)  # [HG, E, T]
            rinv = 1.0 / np.asarray(results[core]["rv"], np.float32)
            for hh in range(HG):
                acc += (poT[hh] * rinv[hh][None, :]).T
        out[b] = acc + bp
    return out, attn


def kernel(x, Wqk, bqk, Wv, bv, Wp, bp, mask):
    runner = _get_runner()
    in_maps = _prep_in_maps(x, Wqk, bqk, Wv, bv, Wp, bp, mask)
    results = runner.execute(in_maps)
    return _assemble(results, bp)


# revision 47
# speedup vs baseline: 1.0859x; 1.0859x over previous
"""Log-sparse attention kernel for 8 TRN2 NeuronCores.

Sharding: batch (4) x head-group (2 groups of 4 heads) = 8 cores, no
collectives.  Per core:
  - causal-conv QK projection as tap-paired K=128 bf16 matmuls (host ships a
    one-shifted duplicate of x^T so two taps share one matmul),
  - transposed scores ST[kj,qi] = K^T Q in bf16 with exact causal widths,
    exp on ScalarE straight out of PSUM (1/sqrt(64) folded into the
    activation scale), multiplicative log-sparse mask in bf16 on VectorE,
  - attention*V with V pre-multiplied by the output projection on the host
    (Wv @ Wp) plus a ones-column, so one matmul chain yields both the
    projected partial output (transposed) and the softmax denominators,
  - attention weights emitted via PE transpose + PSUM copyback + DMA as
    unnormalized bf16; the host applies the 1/rowsum normalization and
    upcast while unsharding.
Emission is software-pipelined across head pairs with descending block
indices so the next pair's score tiles reuse pmt pool slots freed
diagonally by the current pair's output phase.  Only the causal lower
triangle is computed; the PJRT path zero-initializes output buffers so the
upper triangle is never written.
"""

from contextlib import ExitStack

import numpy as np
import ml_dtypes

import concourse.bacc as bacc
import concourse.tile as tile
import concourse.mybir as mybir

B, T, E, H, QL, SUB = 4, 2048, 64, 8, 6, 64
HG = 4          # heads per core
NCORES = 8
NT = T // 128   # 16 blocks of 128 along t/qi/kj

F32 = mybir.dt.float32
F32R = mybir.dt.float32r
BF16 = mybir.dt.bfloat16
BNP = ml_dtypes.bfloat16

def _qi0(kb):
    """First qi column computed for kj-block kb (exact causal start)."""
    return kb * 128


def _class(w):
    for c in (512, 1024, 1536, 2048):
        if w <= c:
            return c
    return 2048


_MT_OFF = []
_o = 0
for _kb in range(NT):
    _MT_OFF.append(_o)
    _o += T - _qi0(_kb)
MT_W = _o  # 20480


def _build_nc():
    nc = bacc.Bacc("TRN2", target_bir_lowering=False, debug=False,
                   num_devices=NCORES)
    xT = nc.dram_tensor("xT", [E + 1, T + QL - 1], BF16, kind="ExternalInput").ap()
    xT2 = nc.dram_tensor("xT2", [128, T + QL - 1], BF16, kind="ExternalInput").ap()
    wqk = nc.dram_tensor("wqk", [128, QL // 2, 2 * HG * E], BF16, kind="ExternalInput").ap()
    cbias = nc.dram_tensor("cbias", [128, 4], F32, kind="ExternalInput").ap()
    wv = nc.dram_tensor("wv", [E + 1, HG * (E + 1)], BF16, kind="ExternalInput").ap()
    mt = nc.dram_tensor("mt", [128, MT_W], BF16, kind="ExternalInput").ap()
    ident = nc.dram_tensor("ident", [128, 128], BF16, kind="ExternalInput").ap()
    w_out = nc.dram_tensor("w", [HG, T, T], BF16, kind="ExternalOutput").ap()
    po_out = nc.dram_tensor("po", [HG, E, T], BF16, kind="ExternalOutput").ap()
    rv_out = nc.dram_tensor("rv", [HG, T], F32, kind="ExternalOutput").ap()

    Exp = mybir.ActivationFunctionType.Exp

    with tile.TileContext(nc) as tc, ExitStack() as ctx:
        const = ctx.enter_context(tc.tile_pool(name="const", bufs=1))
        qk_pool = ctx.enter_context(tc.tile_pool(name="qkT", bufs=1))
        v_pool = ctx.enter_context(tc.tile_pool(name="vall", bufs=1))
        w_pool = ctx.enter_context(tc.tile_pool(name="wrow", bufs=5))
        pmt_pools = {
            kb: ctx.enter_context(
                tc.tile_pool(name=f"pmt{kb}", bufs=(3 if kb >= 6 else 2)))
            for kb in range(NT)
        }
        r_pool = ctx.enter_context(tc.tile_pool(name="rinv", bufs=4))
        po_pool = ctx.enter_context(tc.tile_pool(name="posb", bufs=1))

        # ---- load inputs -------------------------------------------------
        xT_sb = const.tile([E + 1, T + QL - 1], BF16)
        nc.sync.dma_start(xT_sb[:], xT[:])
        xT2_sb = const.tile([128, T + QL - 1], BF16)
        nc.sync.dma_start(xT2_sb[:], xT2[:])
        wqk_sb = const.tile([128, QL // 2, 2 * HG * E], BF16)
        nc.sync.dma_start(wqk_sb[:], wqk[:])
        cbias_sb = const.tile([128, 4], F32)
        nc.sync.dma_start(cbias_sb[:], cbias[:])
        wv_sb = const.tile([E + 1, HG * (E + 1)], BF16)
        nc.sync.dma_start(wv_sb[:], wv[:])
        ident_sb = const.tile([128, 128], BF16)
        nc.sync.dma_start(ident_sb[:], ident[:])
        mt_sb = const.tile([128, MT_W], BF16)
        nc.sync.dma_start(mt_sb[:], mt[:])

        # ---- main psum pools (shared with conv/value via tag) -----------
        main_ctx = ExitStack()
        ps_st = main_ctx.enter_context(
            tc.tile_pool(name="psst", bufs=4, space="PSUM"))
        ps_u = main_ctx.enter_context(
            tc.tile_pool(name="psu", bufs=1, space="PSUM"))
        ps_tr = main_ctx.enter_context(
            tc.tile_pool(name="pstr", bufs=3, space="PSUM"))

        ones_sb = const.tile([1, 1], BF16)
        nc.vector.memset(ones_sb[:], 1.0)
        poT_sb = po_pool.tile([E, HG, T], BF16)

        # qkT in two tiles so heads 0/1 (ct 0,2) never depend on the
        # conv writes for heads 2/3 (ct 1,3).
        qkT02 = qk_pool.tile([128, 2, T], BF16, name="qkT02")
        qkT13 = qk_pool.tile([128, 2, T], BF16, name="qkT13")
        v_sb = v_pool.tile([128, NT, HG * (E + 1)], BF16)

        def emit_conv(ct, tt):
            # 6-tap causal conv for one (channel-tile, t-tile): qk channels
            dst = qkT02 if ct in (0, 2) else qkT13
            di = 0 if ct < 2 else 1
            cps = ps_st.tile([128, 512], F32, name="cps", tag="st")
            for p in range(QL // 2):
                nc.tensor.matmul(
                    cps[:, :],
                    lhsT=wqk_sb[:, p, ct * 128:(ct + 1) * 128],
                    rhs=xT2_sb[:, tt * 512 + 2 * p: tt * 512 + 2 * p + 512],
                    start=(p == 0), stop=(p == QL // 2 - 1),
                )
            nc.vector.tensor_scalar_add(
                dst[:, di, tt * 512:(tt + 1) * 512], cps[:, :],
                cbias_sb[:, ct:ct + 1])

        def emit_value(tb):
            vps = ps_st.tile([128, HG * (E + 1)], F32, name="vps", tag="st")
            nc.tensor.matmul(
                vps[:, :],
                lhsT=xT_sb[:, QL - 1 + tb * 128: QL - 1 + (tb + 1) * 128],
                rhs=wv_sb[:],
                start=True, stop=True,
            )
            nc.vector.tensor_copy(v_sb[:, tb, :], vps[:, :])

        pmt_map = [[None] * NT for _ in range(HG)]

        def emit_A2(pair, kb):
            # Paired heads (2*pair, 2*pair+1) live at partition offsets 0/64:
            # their K=64 score matmuls go to distinct PE row-groups and run
            # concurrently.  ST = K^T Q, exp (1/8 folded), log-sparse mask.
            qk_t = qkT02 if pair == 0 else qkT13
            q0 = _qi0(kb)
            wdt = T - q0
            pa = pmt_pools[kb].tile([128, wdt], BF16, name="pmtA", tag="pmt")
            pb = pmt_pools[kb].tile([128, wdt], BF16, name="pmtB", tag="pmt")
            for p0, pt in ((0, pa), (64, pb)):
                for seg in range(q0, T, 512):
                    nn = min(512, T - seg)
                    st_t = ps_st.tile([128, 512], F32, name="st_t", tag="st")
                    nc.tensor.matmul(
                        st_t[:, 0:nn],
                        lhsT=qk_t[p0:p0 + 64, 1, kb * 128:(kb + 1) * 128],
                        rhs=qk_t[p0:p0 + 64, 0, seg: seg + nn],
                        start=True, stop=True)
                    nc.scalar.activation(
                        pt[:, seg - q0: seg - q0 + nn], st_t[:, 0:nn],
                        Exp, scale=0.125)
            nc.vector.tensor_mul(
                pa[:, :], pa[:, :], mt_sb[:, _MT_OFF[kb]: _MT_OFF[kb] + wdt])
            nc.vector.tensor_mul(
                pb[:, :], pb[:, :], mt_sb[:, _MT_OFF[kb]: _MT_OFF[kb] + wdt])
            pmt_map[2 * pair][kb] = pa
            pmt_map[2 * pair + 1][kb] = pb

        def emit_B(hh, qt):
            # U = V^T P (unnormalized attn out, transposed); ones-column row
            # gives softmax denominators, transposed into [qi,1] via tiny
            # matmuls against a [1,1] ones tile.
            u_t = ps_u.tile([E + 1, 512], F32)
            for kb in range(4 * qt + 4):
                dd = max(0, kb * 128 - qt * 512)
                src0 = qt * 512 + dd - _qi0(kb)
                nc.tensor.matmul(
                    u_t[:, dd:512],
                    lhsT=v_sb[:, kb, hh * (E + 1):(hh + 1) * (E + 1)],
                    rhs=pmt_map[hh][kb][:, src0: src0 + 512 - dd],
                    start=(kb == 0), stop=(kb == 4 * qt + 3))
            nc.vector.tensor_copy(
                poT_sb[:, hh, qt * 512:(qt + 1) * 512], u_t[0:E, :])
            r_chunk = r_pool.tile([1, 512], F32, tag="rrow")
            nc.vector.tensor_copy(r_chunk[0:1, :], u_t[E:E + 1, :])
            nc.sync.dma_start(
                rv_out[hh, qt * 512:(qt + 1) * 512], r_chunk[0:1, :])

        nchunk = [0]

        def emit_C(hh, qb):
            # attn-weight output row qb: PE-transpose back to [qi,kj], copy
            # out of PSUM, DMA the causal prefix (unnormalized bf16; the
            # host applies the softmax denominators).
            wrow = w_pool.tile([128, T], BF16)
            for c0 in range(0, (qb + 1) * 128, 1024):
                cw = min(1024, (qb + 1) * 128 - c0)
                tr = ps_tr.tile([128, 1024], BF16)
                for j in range(0, cw, 128):
                    kbj = (c0 + j) // 128
                    nc.tensor.transpose(
                        tr[:, j:j + 128],
                        pmt_map[hh][kbj][:, qb * 128 - _qi0(kbj):
                                         qb * 128 - _qi0(kbj) + 128],
                        ident_sb[:])
                if nchunk[0] % 3 == 0:
                    nc.scalar.copy(wrow[:, c0:c0 + cw], tr[:, 0:cw])
                else:
                    nc.vector.tensor_copy(wrow[:, c0:c0 + cw], tr[:, 0:cw])
                nchunk[0] += 1
            nc.sync.dma_start(
                w_out[hh, qb * 128:(qb + 1) * 128, 0:(qb + 1) * 128],
                wrow[:, 0:(qb + 1) * 128])

        # Software pipeline: prologue interleaves conv/value with pair-0
        # score tiles; steady state interleaves A2(pair 1) with C(pair 0)
        # descending so pmt pool slots free diagonally.
        prequeue = []
        for tt in range(3, -1, -1):
            prequeue += [("c", 0, tt), ("c", 2, tt), ("v", NT - 1 - (3 - tt), 0)]
        rest = ([("c", 1, tt) for tt in range(4)]
                + [("c", 3, tt) for tt in range(4)]
                + [("v", tb, 0) for tb in range(NT - 4)])
        def emit_item(item):
            if item[0] == "c":
                emit_conv(item[1], item[2])
            else:
                emit_value(item[1])
        for item in prequeue:
            emit_item(item)
        ri = 0
        for kb in range(NT - 1, -1, -1):
            emit_A2(0, kb)
            take = 2 if kb % 2 == 0 else 1
            for _ in range(take):
                if ri < len(rest):
                    emit_item(rest[ri]); ri += 1
        while ri < len(rest):
            emit_item(rest[ri]); ri += 1
        for pair in range(2):
            ha, hb = 2 * pair, 2 * pair + 1
            for qb in range(NT - 1, -1, -1):
                if qb % 4 == 3:
                    emit_B(ha, qb // 4)
                    emit_B(hb, qb // 4)
                emit_C(ha, qb)
                emit_C(hb, qb)
                if pair == 0:
                    emit_A2(1, qb)
        main_ctx.close()

        nc.sync.dma_start(po_out.rearrange("h e t -> e h t"), poT_sb[:])

    nc.compile()
    return nc


# --------------------------------------------------------------------------
# Host-side sharding / unsharding and the cached PJRT runner.
# --------------------------------------------------------------------------

_RUNNER = None


class _Runner:
    def __init__(self):
        import jax
        from jax.experimental.shard_map import shard_map
        from jax.sharding import Mesh, PartitionSpec
        from concourse.bass2jax import (
            _bass_exec_p, install_neuronx_cc_hook, partition_id_tensor)

        install_neuronx_cc_hook()
        nc = self.nc = _build_nc()
        partition_name = (
            nc.partition_id_tensor.name if nc.partition_id_tensor else None)

        in_names, out_names, out_avals, zero_shapes = [], [], [], []
        for alloc in nc.m.functions[0].allocations:
            if not isinstance(alloc, mybir.MemoryLocationSet):
                continue
            name = alloc.memorylocations[0].name
            if alloc.kind == "ExternalInput":
                if name != partition_name:
                    in_names.append(name)
            elif alloc.kind == "ExternalOutput":
                out_names.append(name)
                shape = tuple(alloc.tensor_shape)
                dtype = mybir.dt.np(alloc.dtype)
                out_avals.append(jax.core.ShapedArray(shape, dtype))
                zero_shapes.append((shape, dtype))
        self.in_names = in_names
        self.out_names = out_names
        self.out_avals = out_avals
        self.zero_shapes = zero_shapes
        n_params = len(in_names)
        n_outs = len(out_names)
        all_names = in_names + out_names
        if partition_name is not None:
            all_names = all_names + [partition_name]

        def _body(*args):
            operands = list(args)
            if partition_name is not None:
                operands.append(partition_id_tensor())
            outs = _bass_exec_p.bind(
                *operands,
                out_avals=tuple(out_avals),
                in_names=tuple(all_names),
                out_names=tuple(out_names),
                lowering_input_output_aliases=(),
                sim_require_finite=True,
                sim_require_nnan=True,
                nc=nc,
            )
            return tuple(outs)

        devices = jax.devices()[:NCORES]
        mesh = Mesh(np.asarray(devices), ("core",))
        in_specs = (PartitionSpec("core"),) * (n_params + n_outs)
        out_specs = (PartitionSpec("core"),) * n_outs
        donate = tuple(range(n_params, n_params + n_outs))
        self.sharded = jax.jit(
            shard_map(_body, mesh=mesh, in_specs=in_specs,
                      out_specs=out_specs, check_rep=False),
            donate_argnums=donate, keep_unused=True,
        )

    def execute(self, in_maps):
        concat_in = [
            np.concatenate([np.asarray(m[name]) for m in in_maps], axis=0)
            for name in self.in_names
        ]
        concat_zeros = [
            np.zeros((NCORES * s[0], *s[1:]), d) for (s, d) in self.zero_shapes
        ]
        out_arrs = self.sharded(*concat_in, *concat_zeros)
        return [
            {
                name: np.asarray(out_arrs[i]).reshape(
                    NCORES, *self.out_avals[i].shape)[c]
                for i, name in enumerate(self.out_names)
            }
            for c in range(NCORES)
        ]


def _get_runner():
    global _RUNNER
    if _RUNNER is None:
        _RUNNER = _Runner()
    return _RUNNER


def _prep_in_maps(x, Wqk, bqk, Wv, bv, Wp, bp, mask):
    x = np.asarray(x, np.float32)
    Wqk = np.asarray(Wqk, np.float32)
    bqk = np.asarray(bqk, np.float32)
    Wv = np.asarray(Wv, np.float32)
    bv = np.asarray(bv, np.float32)
    Wp = np.asarray(Wp, np.float32)
    m = np.asarray(mask).reshape(T, T).astype(np.float32)

    # mask, transposed + causally packed (shared by all cores)
    mt_np = np.zeros((128, MT_W), dtype=BNP)
    for kb in range(NT):
        q0 = _qi0(kb)
        blk = m[q0:, kb * 128:(kb + 1) * 128].T  # [128, T-q0]
        mt_np[:, _MT_OFF[kb]: _MT_OFF[kb] + T - q0] = blk.astype(BNP)
    ident_np = np.eye(128, dtype=BNP)

    in_maps = []
    for core in range(NCORES):
        b, g = divmod(core, 2)
        heads = [4 * g + i for i in range(HG)]

        xT_np = np.zeros((E + 1, T + QL - 1), np.float32)
        xT_np[:E, QL - 1:] = x[b].T
        xT_np[E, :] = 1.0

        # channel order: q of the 4 heads (64 each), then k of the 4 heads
        chan = np.concatenate(
            [np.arange(64 * h, 64 * h + 64) for h in heads]
            + [np.arange(512 + 64 * h, 512 + 64 * h + 64) for h in heads])
        # tap-paired conv weights: rows 0:64 = tap 2p, rows 64:128 = tap 2p+1
        wqk_np = np.zeros((128, QL // 2, 2 * HG * E), np.float32)
        wt = Wqk[chan].transpose(1, 2, 0)  # [e, dt, ci]
        for p in range(QL // 2):
            wqk_np[:E, p] = wt[:, 2 * p]
            wqk_np[E:, p] = wt[:, 2 * p + 1]
        cbias_np = bqk[chan].reshape(4, 128).T.astype(np.float32).copy()
        xT2_np = np.zeros((128, T + QL - 1), np.float32)
        xT2_np[:E] = xT_np[:E]
        xT2_np[E:, :-1] = xT_np[:E, 1:]

        wv_np = np.zeros((E + 1, HG * (E + 1)), np.float32)
        for i, h in enumerate(heads):
            wvp = Wv[:, 64 * h: 64 * h + 64] @ Wp[64 * h: 64 * h + 64, :]
            wv_np[:E, i * (E + 1): i * (E + 1) + E] = wvp
            wv_np[E, i * (E + 1): i * (E + 1) + E] = (
                bv[64 * h: 64 * h + 64] @ Wp[64 * h: 64 * h + 64, :])
            wv_np[E, i * (E + 1) + E] = 1.0

        in_maps.append({
            "xT": xT_np.astype(BNP),
            "wqk": wqk_np.astype(BNP),
            "xT2": xT2_np.astype(BNP),
            "cbias": cbias_np,
            "wv": wv_np.astype(BNP),
            "mt": mt_np,
            "ident": ident_np,
        })
    return in_maps


def _assemble(results, bp):
    bp = np.asarray(bp, np.float32)
    attn = np.empty((B, H, T, T), np.float32)
    out = np.empty((B, T, E), np.float32)
    for core in range(NCORES):
        b, g = divmod(core, 2)
        rinv = 1.0 / np.asarray(results[core]["rv"], np.float32)  # [HG, T]
        w = results[core]["w"]                                    # bf16 [HG,T,T]
        for hh in range(HG):
            np.multiply(np.asarray(w[hh], np.float32),
                        rinv[hh][:, None], out=attn[b, 4 * g + hh])
    for b in range(B):
        acc = np.zeros((T, E), np.float32)
        for core in (2 * b, 2 * b + 1):
            poT = np.asarray(results[core]["po"], np.float32)  # [HG, E, T]
            rinv = 1.0 / np.asarray(results[core]["rv"], np.float32)
            for hh in range(HG):
                acc += (poT[hh] * rinv[hh][None, :]).T
        out[b] = acc + bp
    return out, attn


def kernel(x, Wqk, bqk, Wv, bv, Wp, bp, mask):
    runner = _get_runner()
    in_maps = _prep_in_maps(x, Wqk, bqk, Wv, bv, Wp, bp, mask)
    results = runner.execute(in_maps)
    return _assemble(results, bp)


# revision 48
# speedup vs baseline: 1.3779x; 1.2689x over previous
"""Log-sparse attention kernel for 8 TRN2 NeuronCores.

Sharding: batch (4) x head-group (2 groups of 4 heads) = 8 cores, no
collectives.  Per core:
  - causal-conv QK projection as tap-paired K=128 bf16 matmuls (host ships a
    one-shifted duplicate of x^T so two taps share one matmul),
  - transposed scores ST[kj,qi] = K^T Q in bf16 with exact causal widths,
    exp on ScalarE straight out of PSUM (1/sqrt(64) folded into the
    activation scale), multiplicative log-sparse mask in bf16 on VectorE,
  - attention*V with V pre-multiplied by the output projection on the host
    (Wv @ Wp) plus a ones-column, so one matmul chain yields both the
    projected partial output (transposed) and the softmax denominators,
  - attention weights emitted via PE transpose + PSUM copyback + DMA as
    unnormalized bf16; the host applies the 1/rowsum normalization and
    upcast while unsharding.
Emission is software-pipelined across head pairs with descending block
indices so the next pair's score tiles reuse pmt pool slots freed
diagonally by the current pair's output phase.  Only the causal lower
triangle is computed; the PJRT path zero-initializes output buffers so the
upper triangle is never written.
"""

from contextlib import ExitStack

import numpy as np
import ml_dtypes

import concourse.bacc as bacc
import concourse.tile as tile
import concourse.mybir as mybir

B, T, E, H, QL, SUB = 4, 2048, 64, 8, 6, 64
HG = 4          # heads per core
NCORES = 8
NT = T // 128   # 16 blocks of 128 along t/qi/kj

F32 = mybir.dt.float32
F32R = mybir.dt.float32r
BF16 = mybir.dt.bfloat16
BNP = ml_dtypes.bfloat16

def _qi0(kb):
    """First qi column computed for kj-block kb (exact causal start)."""
    return kb * 128


def _class(w):
    for c in (512, 1024, 1536, 2048):
        if w <= c:
            return c
    return 2048


_MT_OFF = []
_o = 0
for _kb in range(NT):
    _MT_OFF.append(_o)
    _o += T - _qi0(_kb)
MT_W = _o  # 20480


def _build_nc():
    nc = bacc.Bacc("TRN2", target_bir_lowering=False, debug=False,
                   num_devices=NCORES)
    xT = nc.dram_tensor("xT", [E + 1, T + QL - 1], BF16, kind="ExternalInput").ap()
    xT2 = nc.dram_tensor("xT2", [128, T + QL - 1], BF16, kind="ExternalInput").ap()
    wqk = nc.dram_tensor("wqk", [128, QL // 2, 2 * HG * E], BF16, kind="ExternalInput").ap()
    cbias = nc.dram_tensor("cbias", [128, 4], F32, kind="ExternalInput").ap()
    wv = nc.dram_tensor("wv", [E + 1, HG * (E + 1)], BF16, kind="ExternalInput").ap()
    mt = nc.dram_tensor("mt", [128, MT_W], BF16, kind="ExternalInput").ap()
    w_out = nc.dram_tensor("w", [HG, NT, 128, T], BF16, kind="ExternalOutput").ap()
    po_out = nc.dram_tensor("po", [HG, E, T], BF16, kind="ExternalOutput").ap()
    rv_out = nc.dram_tensor("rv", [HG, T], F32, kind="ExternalOutput").ap()

    Exp = mybir.ActivationFunctionType.Exp

    with tile.TileContext(nc) as tc, ExitStack() as ctx:
        const = ctx.enter_context(tc.tile_pool(name="const", bufs=1))
        qk_pool = ctx.enter_context(tc.tile_pool(name="qkT", bufs=1))
        v_pool = ctx.enter_context(tc.tile_pool(name="vall", bufs=1))
        pmt_pools = {
            kb: ctx.enter_context(
                tc.tile_pool(name=f"pmt{kb}", bufs=(3 if kb >= 6 else 2)))
            for kb in range(NT)
        }
        r_pool = ctx.enter_context(tc.tile_pool(name="rinv", bufs=4))
        po_pool = ctx.enter_context(tc.tile_pool(name="posb", bufs=1))

        # ---- load inputs -------------------------------------------------
        xT_sb = const.tile([E + 1, T + QL - 1], BF16)
        nc.sync.dma_start(xT_sb[:], xT[:])
        xT2_sb = const.tile([128, T + QL - 1], BF16)
        nc.sync.dma_start(xT2_sb[:], xT2[:])
        wqk_sb = const.tile([128, QL // 2, 2 * HG * E], BF16)
        nc.sync.dma_start(wqk_sb[:], wqk[:])
        cbias_sb = const.tile([128, 4], F32)
        nc.sync.dma_start(cbias_sb[:], cbias[:])
        wv_sb = const.tile([E + 1, HG * (E + 1)], BF16)
        nc.sync.dma_start(wv_sb[:], wv[:])
        mt_sb = const.tile([128, MT_W], BF16)
        nc.sync.dma_start(mt_sb[:], mt[:])

        # ---- main psum pools (shared with conv/value via tag) -----------
        main_ctx = ExitStack()
        ps_st = main_ctx.enter_context(
            tc.tile_pool(name="psst", bufs=6, space="PSUM"))
        ps_u = main_ctx.enter_context(
            tc.tile_pool(name="psu", bufs=2, space="PSUM"))

        poT_sb = po_pool.tile([E, HG, T], BF16)

        # qkT in two tiles so heads 0/1 (ct 0,2) never depend on the
        # conv writes for heads 2/3 (ct 1,3).
        qkT02 = qk_pool.tile([128, 2, T], BF16, name="qkT02")
        qkT13 = qk_pool.tile([128, 2, T], BF16, name="qkT13")
        v_sb = v_pool.tile([128, NT, HG * (E + 1)], BF16)

        def emit_conv(ct, tt):
            # 6-tap causal conv for one (channel-tile, t-tile): qk channels
            dst = qkT02 if ct in (0, 2) else qkT13
            di = 0 if ct < 2 else 1
            cps = ps_st.tile([128, 512], F32, name="cps", tag="st")
            for p in range(QL // 2):
                nc.tensor.matmul(
                    cps[:, :],
                    lhsT=wqk_sb[:, p, ct * 128:(ct + 1) * 128],
                    rhs=xT2_sb[:, tt * 512 + 2 * p: tt * 512 + 2 * p + 512],
                    start=(p == 0), stop=(p == QL // 2 - 1),
                )
            nc.vector.tensor_scalar_add(
                dst[:, di, tt * 512:(tt + 1) * 512], cps[:, :],
                cbias_sb[:, ct:ct + 1])

        def emit_value(tb):
            vps = ps_st.tile([128, HG * (E + 1)], F32, name="vps", tag="st")
            nc.tensor.matmul(
                vps[:, :],
                lhsT=xT_sb[:, QL - 1 + tb * 128: QL - 1 + (tb + 1) * 128],
                rhs=wv_sb[:],
                start=True, stop=True,
            )
            nc.vector.tensor_copy(v_sb[:, tb, :], vps[:, :])

        pmt_map = [[None] * NT for _ in range(HG)]

        def emit_A2(pair, kb):
            # Paired heads (2*pair, 2*pair+1) live at partition offsets 0/64:
            # their K=64 score matmuls go to distinct PE row-groups and run
            # concurrently.  ST = K^T Q, exp (1/8 folded), log-sparse mask.
            qk_t = qkT02 if pair == 0 else qkT13
            q0 = _qi0(kb)
            wdt = T - q0
            pa = pmt_pools[kb].tile([128, wdt], BF16, name="pmtA", tag="pmt")
            pb = pmt_pools[kb].tile([128, wdt], BF16, name="pmtB", tag="pmt")
            for p0, pt in ((0, pa), (64, pb)):
                for seg in range(q0, T, 512):
                    nn = min(512, T - seg)
                    st_t = ps_st.tile([128, 512], F32, name="st_t", tag="st")
                    nc.tensor.matmul(
                        st_t[:, 0:nn],
                        lhsT=qk_t[p0:p0 + 64, 1, kb * 128:(kb + 1) * 128],
                        rhs=qk_t[p0:p0 + 64, 0, seg: seg + nn],
                        start=True, stop=True)
                    nc.scalar.activation(
                        pt[:, seg - q0: seg - q0 + nn], st_t[:, 0:nn],
                        Exp, scale=0.125)
            nc.vector.tensor_mul(
                pa[:, :], pa[:, :], mt_sb[:, _MT_OFF[kb]: _MT_OFF[kb] + wdt])
            nc.vector.tensor_mul(
                pb[:, :], pb[:, :], mt_sb[:, _MT_OFF[kb]: _MT_OFF[kb] + wdt])
            nc.sync.dma_start(w_out[2 * pair, kb, :, q0:T], pa[:, :])
            nc.sync.dma_start(w_out[2 * pair + 1, kb, :, q0:T], pb[:, :])
            pmt_map[2 * pair][kb] = pa
            pmt_map[2 * pair + 1][kb] = pb

        def emit_B(hh, qt):
            # U = V^T P (unnormalized attn out, transposed); ones-column row
            # gives softmax denominators, transposed into [qi,1] via tiny
            # matmuls against a [1,1] ones tile.
            u_t = ps_u.tile([E + 1, 512], F32)
            for kb in range(4 * qt + 4):
                dd = max(0, kb * 128 - qt * 512)
                src0 = qt * 512 + dd - _qi0(kb)
                nc.tensor.matmul(
                    u_t[:, dd:512],
                    lhsT=v_sb[:, kb, hh * (E + 1):(hh + 1) * (E + 1)],
                    rhs=pmt_map[hh][kb][:, src0: src0 + 512 - dd],
                    start=(kb == 0), stop=(kb == 4 * qt + 3))
            nc.vector.tensor_copy(
                poT_sb[:, hh, qt * 512:(qt + 1) * 512], u_t[0:E, :])
            r_chunk = r_pool.tile([1, 512], F32, tag="rrow")
            nc.vector.tensor_copy(r_chunk[0:1, :], u_t[E:E + 1, :])
            nc.sync.dma_start(
                rv_out[hh, qt * 512:(qt + 1) * 512], r_chunk[0:1, :])

        # Software pipeline: prologue interleaves conv/value with pair-0
        # score tiles; steady state interleaves A2(pair 1) with C(pair 0)
        # descending so pmt pool slots free diagonally.
        prequeue = []
        for tt in range(3, -1, -1):
            prequeue += [("c", 0, tt), ("c", 2, tt), ("v", NT - 1 - (3 - tt), 0)]
        rest = ([("c", 1, tt) for tt in range(4)]
                + [("c", 3, tt) for tt in range(4)]
                + [("v", tb, 0) for tb in range(NT - 4)])
        def emit_item(item):
            if item[0] == "c":
                emit_conv(item[1], item[2])
            else:
                emit_value(item[1])
        for item in prequeue:
            emit_item(item)
        ri = 0
        for kb in range(NT - 1, -1, -1):
            emit_A2(0, kb)
            take = 2 if kb % 2 == 0 else 1
            for _ in range(take):
                if ri < len(rest):
                    emit_item(rest[ri]); ri += 1
        while ri < len(rest):
            emit_item(rest[ri]); ri += 1
        for pair in range(2):
            ha, hb = 2 * pair, 2 * pair + 1
            for qb in range(NT - 1, -1, -1):
                if qb % 4 == 3:
                    emit_B(ha, qb // 4)
                    emit_B(hb, qb // 4)
                if pair == 0:
                    emit_A2(1, qb)
        main_ctx.close()

        nc.sync.dma_start(po_out.rearrange("h e t -> e h t"), poT_sb[:])

    nc.compile()
    return nc


# --------------------------------------------------------------------------
# Host-side sharding / unsharding and the cached PJRT runner.
# --------------------------------------------------------------------------

_RUNNER = None


class _Runner:
    def __init__(self):
        import jax
        from jax.experimental.shard_map import shard_map
        from jax.sharding import Mesh, PartitionSpec
        from concourse.bass2jax import (
            _bass_exec_p, install_neuronx_cc_hook, partition_id_tensor)

        install_neuronx_cc_hook()
        nc = self.nc = _build_nc()
        partition_name = (
            nc.partition_id_tensor.name if nc.partition_id_tensor else None)

        in_names, out_names, out_avals, zero_shapes = [], [], [], []
        for alloc in nc.m.functions[0].allocations:
            if not isinstance(alloc, mybir.MemoryLocationSet):
                continue
            name = alloc.memorylocations[0].name
            if alloc.kind == "ExternalInput":
                if name != partition_name:
                    in_names.append(name)
            elif alloc.kind == "ExternalOutput":
                out_names.append(name)
                shape = tuple(alloc.tensor_shape)
                dtype = mybir.dt.np(alloc.dtype)
                out_avals.append(jax.core.ShapedArray(shape, dtype))
                zero_shapes.append((shape, dtype))
        self.in_names = in_names
        self.out_names = out_names
        self.out_avals = out_avals
        self.zero_shapes = zero_shapes
        n_params = len(in_names)
        n_outs = len(out_names)
        all_names = in_names + out_names
        if partition_name is not None:
            all_names = all_names + [partition_name]

        def _body(*args):
            operands = list(args)
            if partition_name is not None:
                operands.append(partition_id_tensor())
            outs = _bass_exec_p.bind(
                *operands,
                out_avals=tuple(out_avals),
                in_names=tuple(all_names),
                out_names=tuple(out_names),
                lowering_input_output_aliases=(),
                sim_require_finite=True,
                sim_require_nnan=True,
                nc=nc,
            )
            return tuple(outs)

        devices = jax.devices()[:NCORES]
        mesh = Mesh(np.asarray(devices), ("core",))
        in_specs = (PartitionSpec("core"),) * (n_params + n_outs)
        out_specs = (PartitionSpec("core"),) * n_outs
        donate = tuple(range(n_params, n_params + n_outs))
        self.sharded = jax.jit(
            shard_map(_body, mesh=mesh, in_specs=in_specs,
                      out_specs=out_specs, check_rep=False),
            donate_argnums=donate, keep_unused=True,
        )

    def execute(self, in_maps):
        concat_in = [
            np.concatenate([np.asarray(m[name]) for m in in_maps], axis=0)
            for name in self.in_names
        ]
        concat_zeros = [
            np.zeros((NCORES * s[0], *s[1:]), d) for (s, d) in self.zero_shapes
        ]
        out_arrs = self.sharded(*concat_in, *concat_zeros)
        return [
            {
                name: np.asarray(out_arrs[i]).reshape(
                    NCORES, *self.out_avals[i].shape)[c]
                for i, name in enumerate(self.out_names)
            }
            for c in range(NCORES)
        ]


def _get_runner():
    global _RUNNER
    if _RUNNER is None:
        _RUNNER = _Runner()
    return _RUNNER


def _prep_in_maps(x, Wqk, bqk, Wv, bv, Wp, bp, mask):
    x = np.asarray(x, np.float32)
    Wqk = np.asarray(Wqk, np.float32)
    bqk = np.asarray(bqk, np.float32)
    Wv = np.asarray(Wv, np.float32)
    bv = np.asarray(bv, np.float32)
    Wp = np.asarray(Wp, np.float32)
    m = np.asarray(mask).reshape(T, T).astype(np.float32)

    # mask, transposed + causally packed (shared by all cores)
    mt_np = np.zeros((128, MT_W), dtype=BNP)
    for kb in range(NT):
        q0 = _qi0(kb)
        blk = m[q0:, kb * 128:(kb + 1) * 128].T  # [128, T-q0]
        mt_np[:, _MT_OFF[kb]: _MT_OFF[kb] + T - q0] = blk.astype(BNP)
    ident_np = np.eye(128, dtype=BNP)

    in_maps = []
    for core in range(NCORES):
        b, g = divmod(core, 2)
        heads = [4 * g + i for i in range(HG)]

        xT_np = np.zeros((E + 1, T + QL - 1), np.float32)
        xT_np[:E, QL - 1:] = x[b].T
        xT_np[E, :] = 1.0

        # channel order: q of the 4 heads (64 each), then k of the 4 heads
        chan = np.concatenate(
            [np.arange(64 * h, 64 * h + 64) for h in heads]
            + [np.arange(512 + 64 * h, 512 + 64 * h + 64) for h in heads])
        # tap-paired conv weights: rows 0:64 = tap 2p, rows 64:128 = tap 2p+1
        wqk_np = np.zeros((128, QL // 2, 2 * HG * E), np.float32)
        wt = Wqk[chan].transpose(1, 2, 0)  # [e, dt, ci]
        for p in range(QL // 2):
            wqk_np[:E, p] = wt[:, 2 * p]
            wqk_np[E:, p] = wt[:, 2 * p + 1]
        cbias_np = bqk[chan].reshape(4, 128).T.astype(np.float32).copy()
        xT2_np = np.zeros((128, T + QL - 1), np.float32)
        xT2_np[:E] = xT_np[:E]
        xT2_np[E:, :-1] = xT_np[:E, 1:]

        wv_np = np.zeros((E + 1, HG * (E + 1)), np.float32)
        for i, h in enumerate(heads):
            wvp = Wv[:, 64 * h: 64 * h + 64] @ Wp[64 * h: 64 * h + 64, :]
            wv_np[:E, i * (E + 1): i * (E + 1) + E] = wvp
            wv_np[E, i * (E + 1): i * (E + 1) + E] = (
                bv[64 * h: 64 * h + 64] @ Wp[64 * h: 64 * h + 64, :])
            wv_np[E, i * (E + 1) + E] = 1.0

        in_maps.append({
            "xT": xT_np.astype(BNP),
            "wqk": wqk_np.astype(BNP),
            "xT2": xT2_np.astype(BNP),
            "cbias": cbias_np,
            "wv": wv_np.astype(BNP),
            "mt": mt_np,
        })
    return in_maps


def _assemble(results, bp):
    bp = np.asarray(bp, np.float32)
    attn = np.zeros((B, H, T, T), np.float32)
    out = np.empty((B, T, E), np.float32)
    for core in range(NCORES):
        b, g = divmod(core, 2)
        rinv = 1.0 / np.asarray(results[core]["rv"], np.float32)  # [HG, T]
        w = results[core]["w"]                    # bf16 [HG, NT, 128, T]
        for hh in range(HG):
            a = attn[b, 4 * g + hh]
            for kb in range(NT):
                q0 = 128 * kb
                a[q0:, q0:q0 + 128] = w[hh, kb, :, q0:].T
            a *= rinv[hh][:, None]
    for b in range(B):
        acc = np.zeros((T, E), np.float32)
        for core in (2 * b, 2 * b + 1):
            poT = np.asarray(results[core]["po"], np.float32)  # [HG, E, T]
            rinv = 1.0 / np.asarray(results[core]["rv"], np.float32)
            for hh in range(HG):
                acc += (poT[hh] * rinv[hh][None, :]).T
        out[b] = acc + bp
    return out, attn


def kernel(x, Wqk, bqk, Wv, bv, Wp, bp, mask):
    runner = _get_runner()
    in_maps = _prep_in_maps(x, Wqk, bqk, Wv, bv, Wp, bp, mask)
    results = runner.execute(in_maps)
    return _assemble(results, bp)


# revision 49
# speedup vs baseline: 1.3808x; 1.0021x over previous
"""Log-sparse attention kernel for 8 TRN2 NeuronCores.

Sharding: batch (4) x head-group (2 groups of 4 heads) = 8 cores, no
collectives.  Per core:
  - causal-conv QK projection as tap-paired K=128 bf16 matmuls (host ships a
    one-shifted duplicate of x^T so two taps share one matmul),
  - transposed scores ST[kj,qi] = K^T Q in bf16 with exact causal widths,
    exp on ScalarE straight out of PSUM (1/sqrt(64) folded into the
    activation scale), multiplicative log-sparse mask in bf16 on VectorE,
  - attention*V with V pre-multiplied by the output projection on the host
    (Wv @ Wp) plus a ones-column, so one matmul chain yields both the
    projected partial output (transposed) and the softmax denominators,
  - attention weights leave the device as unnormalized bf16 tiles in the
    transposed [kj, qi] layout they were computed in; the host applies the
    layout permute, 1/rowsum normalization and f32 upcast while unsharding.
Emission is software-pipelined across head pairs with descending block
indices so the next pair's score tiles reuse pmt pool slots as the current
pair's attention*V phase releases them.  Only the causal lower triangle is
computed.
"""

from contextlib import ExitStack

import numpy as np
import ml_dtypes

import concourse.bacc as bacc
import concourse.tile as tile
import concourse.mybir as mybir

B, T, E, H, QL, SUB = 4, 2048, 64, 8, 6, 64
HG = 4          # heads per core
NCORES = 8
NT = T // 128   # 16 blocks of 128 along t/qi/kj

F32 = mybir.dt.float32
F32R = mybir.dt.float32r
BF16 = mybir.dt.bfloat16
BNP = ml_dtypes.bfloat16

def _qi0(kb):
    """First qi column computed for kj-block kb (exact causal start)."""
    return kb * 128


def _class(w):
    for c in (512, 1024, 1536, 2048):
        if w <= c:
            return c
    return 2048


_MT_OFF = []
_o = 0
for _kb in range(NT):
    _MT_OFF.append(_o)
    _o += T - _qi0(_kb)
MT_W = _o  # 20480


def _build_nc():
    nc = bacc.Bacc("TRN2", target_bir_lowering=False, debug=False,
                   num_devices=NCORES)
    xT = nc.dram_tensor("xT", [E + 1, T + QL - 1], BF16, kind="ExternalInput").ap()
    xT2 = nc.dram_tensor("xT2", [128, T + QL - 1], BF16, kind="ExternalInput").ap()
    wqk = nc.dram_tensor("wqk", [128, QL // 2, 2 * HG * E], BF16, kind="ExternalInput").ap()
    cbias = nc.dram_tensor("cbias", [128, 4], F32, kind="ExternalInput").ap()
    wv = nc.dram_tensor("wv", [E + 1, HG * (E + 1)], BF16, kind="ExternalInput").ap()
    mt = nc.dram_tensor("mt", [128, MT_W], BF16, kind="ExternalInput").ap()
    w_out = nc.dram_tensor("w", [HG, NT, 128, T], BF16, kind="ExternalOutput").ap()
    po_out = nc.dram_tensor("po", [HG, E, T], BF16, kind="ExternalOutput").ap()
    rv_out = nc.dram_tensor("rv", [HG, T], F32, kind="ExternalOutput").ap()

    Exp = mybir.ActivationFunctionType.Exp

    with tile.TileContext(nc) as tc, ExitStack() as ctx:
        const = ctx.enter_context(tc.tile_pool(name="const", bufs=1))
        qk_pool = ctx.enter_context(tc.tile_pool(name="qkT", bufs=1))
        v_pool = ctx.enter_context(tc.tile_pool(name="vall", bufs=1))
        pmt_pools = {
            kb: ctx.enter_context(
                tc.tile_pool(name=f"pmt{kb}", bufs=(3 if kb >= 6 else 2)))
            for kb in range(NT)
        }
        r_pool = ctx.enter_context(tc.tile_pool(name="rinv", bufs=4))
        po_pool = ctx.enter_context(tc.tile_pool(name="posb", bufs=1))

        # ---- load inputs -------------------------------------------------
        xT_sb = const.tile([E + 1, T + QL - 1], BF16)
        nc.sync.dma_start(xT_sb[:], xT[:])
        xT2_sb = const.tile([128, T + QL - 1], BF16)
        nc.sync.dma_start(xT2_sb[:], xT2[:])
        wqk_sb = const.tile([128, QL // 2, 2 * HG * E], BF16)
        nc.sync.dma_start(wqk_sb[:], wqk[:])
        cbias_sb = const.tile([128, 4], F32)
        nc.sync.dma_start(cbias_sb[:], cbias[:])
        wv_sb = const.tile([E + 1, HG * (E + 1)], BF16)
        nc.sync.dma_start(wv_sb[:], wv[:])
        mt_sb = const.tile([128, MT_W], BF16)
        nc.sync.dma_start(mt_sb[:], mt[:])

        # ---- main psum pools (shared with conv/value via tag) -----------
        main_ctx = ExitStack()
        ps_st = main_ctx.enter_context(
            tc.tile_pool(name="psst", bufs=6, space="PSUM"))
        ps_u = main_ctx.enter_context(
            tc.tile_pool(name="psu", bufs=2, space="PSUM"))

        poT_sb = po_pool.tile([E, HG, T], BF16)

        # qkT in two tiles so heads 0/1 (ct 0,2) never depend on the
        # conv writes for heads 2/3 (ct 1,3).
        qkT02 = qk_pool.tile([128, 2, T], BF16, name="qkT02")
        qkT13 = qk_pool.tile([128, 2, T], BF16, name="qkT13")
        v_sb = v_pool.tile([128, NT, HG * (E + 1)], BF16)

        def emit_conv(ct, tt):
            # 6-tap causal conv for one (channel-tile, t-tile): qk channels
            dst = qkT02 if ct in (0, 2) else qkT13
            di = 0 if ct < 2 else 1
            cps = ps_st.tile([128, 512], F32, name="cps", tag="st")
            for p in range(QL // 2):
                nc.tensor.matmul(
                    cps[:, :],
                    lhsT=wqk_sb[:, p, ct * 128:(ct + 1) * 128],
                    rhs=xT2_sb[:, tt * 512 + 2 * p: tt * 512 + 2 * p + 512],
                    start=(p == 0), stop=(p == QL // 2 - 1),
                )
            nc.vector.tensor_scalar_add(
                dst[:, di, tt * 512:(tt + 1) * 512], cps[:, :],
                cbias_sb[:, ct:ct + 1])

        def emit_value(tb):
            vps = ps_st.tile([128, HG * (E + 1)], F32, name="vps", tag="st")
            nc.tensor.matmul(
                vps[:, :],
                lhsT=xT_sb[:, QL - 1 + tb * 128: QL - 1 + (tb + 1) * 128],
                rhs=wv_sb[:],
                start=True, stop=True,
            )
            nc.vector.tensor_copy(v_sb[:, tb, :], vps[:, :])

        pmt_map = [[None] * NT for _ in range(HG)]

        def emit_A2(pair, kb):
            # Paired heads (2*pair, 2*pair+1) live at partition offsets 0/64:
            # their K=64 score matmuls go to distinct PE row-groups and run
            # concurrently.  ST = K^T Q, exp (1/8 folded), log-sparse mask.
            qk_t = qkT02 if pair == 0 else qkT13
            q0 = _qi0(kb)
            wdt = T - q0
            pa = pmt_pools[kb].tile([128, wdt], BF16, name="pmtA", tag="pmt")
            pb = pmt_pools[kb].tile([128, wdt], BF16, name="pmtB", tag="pmt")
            for p0, pt in ((0, pa), (64, pb)):
                for seg in range(q0, T, 512):
                    nn = min(512, T - seg)
                    st_t = ps_st.tile([128, 512], F32, name="st_t", tag="st")
                    nc.tensor.matmul(
                        st_t[:, 0:nn],
                        lhsT=qk_t[p0:p0 + 64, 1, kb * 128:(kb + 1) * 128],
                        rhs=qk_t[p0:p0 + 64, 0, seg: seg + nn],
                        start=True, stop=True)
                    nc.scalar.activation(
                        pt[:, seg - q0: seg - q0 + nn], st_t[:, 0:nn],
                        Exp, scale=0.125)
            nc.vector.tensor_mul(
                pa[:, :], pa[:, :], mt_sb[:, _MT_OFF[kb]: _MT_OFF[kb] + wdt])
            nc.vector.tensor_mul(
                pb[:, :], pb[:, :], mt_sb[:, _MT_OFF[kb]: _MT_OFF[kb] + wdt])
            nc.sync.dma_start(w_out[2 * pair, kb, :, q0:T], pa[:, :])
            nc.sync.dma_start(w_out[2 * pair + 1, kb, :, q0:T], pb[:, :])
            pmt_map[2 * pair][kb] = pa
            pmt_map[2 * pair + 1][kb] = pb

        def emit_B(hh, qt):
            # U = V^T P (unnormalized attn out, transposed); ones-column row
            # gives softmax denominators, transposed into [qi,1] via tiny
            # matmuls against a [1,1] ones tile.
            u_t = ps_u.tile([E + 1, 512], F32)
            for kb in range(4 * qt + 4):
                dd = max(0, kb * 128 - qt * 512)
                src0 = qt * 512 + dd - _qi0(kb)
                nc.tensor.matmul(
                    u_t[:, dd:512],
                    lhsT=v_sb[:, kb, hh * (E + 1):(hh + 1) * (E + 1)],
                    rhs=pmt_map[hh][kb][:, src0: src0 + 512 - dd],
                    start=(kb == 0), stop=(kb == 4 * qt + 3))
            nc.vector.tensor_copy(
                poT_sb[:, hh, qt * 512:(qt + 1) * 512], u_t[0:E, :])
            r_chunk = r_pool.tile([1, 512], F32, tag="rrow")
            nc.vector.tensor_copy(r_chunk[0:1, :], u_t[E:E + 1, :])
            nc.sync.dma_start(
                rv_out[hh, qt * 512:(qt + 1) * 512], r_chunk[0:1, :])

        # Software pipeline: prologue interleaves conv/value with pair-0
        # score tiles; steady state interleaves A2(pair 1) with C(pair 0)
        # descending so pmt pool slots free diagonally.
        prequeue = []
        for tt in range(3, -1, -1):
            prequeue += [("c", 0, tt), ("c", 2, tt), ("v", NT - 1 - (3 - tt), 0)]
        rest = ([("c", 1, tt) for tt in range(4)]
                + [("c", 3, tt) for tt in range(4)]
                + [("v", tb, 0) for tb in range(NT - 4)])
        def emit_item(item):
            if item[0] == "c":
                emit_conv(item[1], item[2])
            else:
                emit_value(item[1])
        for item in prequeue:
            emit_item(item)
        ri = 0
        for kb in range(NT - 1, -1, -1):
            emit_A2(0, kb)
            take = 2 if kb % 2 == 0 else 1
            for _ in range(take):
                if ri < len(rest):
                    emit_item(rest[ri]); ri += 1
        while ri < len(rest):
            emit_item(rest[ri]); ri += 1
        for pair in range(2):
            ha, hb = 2 * pair, 2 * pair + 1
            for qb in range(NT - 1, -1, -1):
                if qb % 4 == 3:
                    emit_B(ha, qb // 4)
                    emit_B(hb, qb // 4)
                if pair == 0:
                    emit_A2(1, qb)
        main_ctx.close()

        nc.sync.dma_start(po_out.rearrange("h e t -> e h t"), poT_sb[:])

    nc.compile()
    return nc


# --------------------------------------------------------------------------
# Host-side sharding / unsharding and the cached PJRT runner.
# --------------------------------------------------------------------------

_RUNNER = None


class _Runner:
    def __init__(self):
        import jax
        from jax.experimental.shard_map import shard_map
        from jax.sharding import Mesh, PartitionSpec
        from concourse.bass2jax import (
            _bass_exec_p, install_neuronx_cc_hook, partition_id_tensor)

        install_neuronx_cc_hook()
        nc = self.nc = _build_nc()
        partition_name = (
            nc.partition_id_tensor.name if nc.partition_id_tensor else None)

        in_names, out_names, out_avals, zero_shapes = [], [], [], []
        for alloc in nc.m.functions[0].allocations:
            if not isinstance(alloc, mybir.MemoryLocationSet):
                continue
            name = alloc.memorylocations[0].name
            if alloc.kind == "ExternalInput":
                if name != partition_name:
                    in_names.append(name)
            elif alloc.kind == "ExternalOutput":
                out_names.append(name)
                shape = tuple(alloc.tensor_shape)
                dtype = mybir.dt.np(alloc.dtype)
                out_avals.append(jax.core.ShapedArray(shape, dtype))
                zero_shapes.append((shape, dtype))
        self.in_names = in_names
        self.out_names = out_names
        self.out_avals = out_avals
        self.zero_shapes = zero_shapes
        n_params = len(in_names)
        n_outs = len(out_names)
        all_names = in_names + out_names
        if partition_name is not None:
            all_names = all_names + [partition_name]

        def _body(*args):
            operands = list(args)
            if partition_name is not None:
                operands.append(partition_id_tensor())
            outs = _bass_exec_p.bind(
                *operands,
                out_avals=tuple(out_avals),
                in_names=tuple(all_names),
                out_names=tuple(out_names),
                lowering_input_output_aliases=(),
                sim_require_finite=True,
                sim_require_nnan=True,
                nc=nc,
            )
            return tuple(outs)

        devices = jax.devices()[:NCORES]
        mesh = Mesh(np.asarray(devices), ("core",))
        in_specs = (PartitionSpec("core"),) * (n_params + n_outs)
        out_specs = (PartitionSpec("core"),) * n_outs
        donate = tuple(range(n_params, n_params + n_outs))
        self.sharded = jax.jit(
            shard_map(_body, mesh=mesh, in_specs=in_specs,
                      out_specs=out_specs, check_rep=False),
            donate_argnums=donate, keep_unused=True,
        )

    def execute(self, in_maps):
        concat_in = [
            np.concatenate([np.asarray(m[name]) for m in in_maps], axis=0)
            for name in self.in_names
        ]
        concat_zeros = [
            np.zeros((NCORES * s[0], *s[1:]), d) for (s, d) in self.zero_shapes
        ]
        out_arrs = self.sharded(*concat_in, *concat_zeros)
        return [
            {
                name: np.asarray(out_arrs[i]).reshape(
                    NCORES, *self.out_avals[i].shape)[c]
                for i, name in enumerate(self.out_names)
            }
            for c in range(NCORES)
        ]


def _get_runner():
    global _RUNNER
    if _RUNNER is None:
        _RUNNER = _Runner()
    return _RUNNER


def _prep_in_maps(x, Wqk, bqk, Wv, bv, Wp, bp, mask):
    x = np.asarray(x, np.float32)
    Wqk = np.asarray(Wqk, np.float32)
    bqk = np.asarray(bqk, np.float32)
    Wv = np.asarray(Wv, np.float32)
    bv = np.asarray(bv, np.float32)
    Wp = np.asarray(Wp, np.float32)
    m = np.asarray(mask).reshape(T, T).astype(np.float32)

    # mask, transposed + causally packed (shared by all cores)
    mt_np = np.zeros((128, MT_W), dtype=BNP)
    for kb in range(NT):
        q0 = _qi0(kb)
        blk = m[q0:, kb * 128:(kb + 1) * 128].T  # [128, T-q0]
        mt_np[:, _MT_OFF[kb]: _MT_OFF[kb] + T - q0] = blk.astype(BNP)
    ident_np = np.eye(128, dtype=BNP)

    in_maps = []
    for core in range(NCORES):
        b, g = divmod(core, 2)
        heads = [4 * g + i for i in range(HG)]

        xT_np = np.zeros((E + 1, T + QL - 1), np.float32)
        xT_np[:E, QL - 1:] = x[b].T
        xT_np[E, :] = 1.0

        # channel order: q of the 4 heads (64 each), then k of the 4 heads
        chan = np.concatenate(
            [np.arange(64 * h, 64 * h + 64) for h in heads]
            + [np.arange(512 + 64 * h, 512 + 64 * h + 64) for h in heads])
        # tap-paired conv weights: rows 0:64 = tap 2p, rows 64:128 = tap 2p+1
        wqk_np = np.zeros((128, QL // 2, 2 * HG * E), np.float32)
        wt = Wqk[chan].transpose(1, 2, 0)  # [e, dt, ci]
        for p in range(QL // 2):
            wqk_np[:E, p] = wt[:, 2 * p]
            wqk_np[E:, p] = wt[:, 2 * p + 1]
        cbias_np = bqk[chan].reshape(4, 128).T.astype(np.float32).copy()
        xT2_np = np.zeros((128, T + QL - 1), np.float32)
        xT2_np[:E] = xT_np[:E]
        xT2_np[E:, :-1] = xT_np[:E, 1:]

        wv_np = np.zeros((E + 1, HG * (E + 1)), np.float32)
        for i, h in enumerate(heads):
            wvp = Wv[:, 64 * h: 64 * h + 64] @ Wp[64 * h: 64 * h + 64, :]
            wv_np[:E, i * (E + 1): i * (E + 1) + E] = wvp
            wv_np[E, i * (E + 1): i * (E + 1) + E] = (
                bv[64 * h: 64 * h + 64] @ Wp[64 * h: 64 * h + 64, :])
            wv_np[E, i * (E + 1) + E] = 1.0

        in_maps.append({
            "xT": xT_np.astype(BNP),
            "wqk": wqk_np.astype(BNP),
            "xT2": xT2_np.astype(BNP),
            "cbias": cbias_np,
            "wv": wv_np.astype(BNP),
            "mt": mt_np,
        })
    return in_maps


def _assemble(results, bp):
    bp = np.asarray(bp, np.float32)
    attn = np.zeros((B, H, T, T), np.float32)
    out = np.empty((B, T, E), np.float32)
    for core in range(NCORES):
        b, g = divmod(core, 2)
        rinv = 1.0 / np.asarray(results[core]["rv"], np.float32)  # [HG, T]
        w = results[core]["w"]                    # bf16 [HG, NT, 128, T]
        for hh in range(HG):
            a = attn[b, 4 * g + hh]
            for kb in range(NT):
                q0 = 128 * kb
                a[q0:, q0:q0 + 128] = w[hh, kb, :, q0:].T
            a *= rinv[hh][:, None]
    for b in range(B):
        acc = np.zeros((T, E), np.float32)
        for core in (2 * b, 2 * b + 1):
            poT = np.asarray(results[core]["po"], np.float32)  # [HG, E, T]
            rinv = 1.0 / np.asarray(results[core]["rv"], np.float32)
            for hh in range(HG):
                acc += (poT[hh] * rinv[hh][None, :]).T
        out[b] = acc + bp
    return out, attn


def kernel(x, Wqk, bqk, Wv, bv, Wp, bp, mask):
    runner = _get_runner()
    in_maps = _prep_in_maps(x, Wqk, bqk, Wv, bv, Wp, bp, mask)
    results = runner.execute(in_maps)
    return _assemble(results, bp)


# revision 50
# speedup vs baseline: 1.3920x; 1.0081x over previous
"""Log-sparse attention kernel for 8 TRN2 NeuronCores.

Sharding: batch (4) x head-group (2 groups of 4 heads) = 8 cores, no
collectives.  Per core:
  - causal-conv QK projection as tap-paired K=128 bf16 matmuls (host ships a
    one-shifted duplicate of x^T so two taps share one matmul),
  - transposed scores ST[kj,qi] = K^T Q in bf16 with exact causal widths,
    exp on ScalarE straight out of PSUM (1/sqrt(64) folded into the
    activation scale), multiplicative log-sparse mask in bf16 on VectorE,
  - attention*V with V pre-multiplied by the output projection on the host
    (Wv @ Wp) plus a ones-column, so one matmul chain yields both the
    projected partial output (transposed) and the softmax denominators,
  - attention weights leave the device as unnormalized bf16 tiles in the
    transposed [kj, qi] layout they were computed in; the host applies the
    layout permute, 1/rowsum normalization and f32 upcast while unsharding.
Emission is software-pipelined across head pairs with descending block
indices so the next pair's score tiles reuse pmt pool slots as the current
pair's attention*V phase releases them.  Only the causal lower triangle is
computed.
"""

from contextlib import ExitStack

import numpy as np
import ml_dtypes

import concourse.bacc as bacc
import concourse.tile as tile
import concourse.mybir as mybir

B, T, E, H, QL, SUB = 4, 2048, 64, 8, 6, 64
HG = 4          # heads per core
NCORES = 8
NT = T // 128   # 16 blocks of 128 along t/qi/kj

F32 = mybir.dt.float32
F32R = mybir.dt.float32r
BF16 = mybir.dt.bfloat16
BNP = ml_dtypes.bfloat16

def _qi0(kb):
    """First qi column computed for kj-block kb (exact causal start)."""
    return kb * 128


def _class(w):
    for c in (512, 1024, 1536, 2048):
        if w <= c:
            return c
    return 2048


_MT_OFF = []
_o = 0
for _kb in range(NT):
    _MT_OFF.append(_o)
    _o += T - _qi0(_kb)
MT_W = _o  # 20480


def _build_nc():
    nc = bacc.Bacc("TRN2", target_bir_lowering=False, debug=False,
                   num_devices=NCORES)
    xT = nc.dram_tensor("xT", [E + 1, T + QL - 1], BF16, kind="ExternalInput").ap()
    xT2 = nc.dram_tensor("xT2", [128, T + QL - 1], BF16, kind="ExternalInput").ap()
    wqk = nc.dram_tensor("wqk", [128, QL // 2, 2 * HG * E], BF16, kind="ExternalInput").ap()
    cbias = nc.dram_tensor("cbias", [128, 4], F32, kind="ExternalInput").ap()
    wv = nc.dram_tensor("wv", [E + 1, HG * (E + 1)], BF16, kind="ExternalInput").ap()
    mt = nc.dram_tensor("mt", [128, MT_W], BF16, kind="ExternalInput").ap()
    w_out = nc.dram_tensor("w", [HG, NT, 128, T], BF16, kind="ExternalOutput").ap()
    po_out = nc.dram_tensor("po", [HG, E, T], BF16, kind="ExternalOutput").ap()
    rv_out = nc.dram_tensor("rv", [HG, T], F32, kind="ExternalOutput").ap()

    Exp = mybir.ActivationFunctionType.Exp

    with tile.TileContext(nc) as tc, ExitStack() as ctx:
        const = ctx.enter_context(tc.tile_pool(name="const", bufs=1))
        qk_pool = ctx.enter_context(tc.tile_pool(name="qkT", bufs=1))
        v_pool = ctx.enter_context(tc.tile_pool(name="vall", bufs=1))
        pmt_pools = {
            kb: ctx.enter_context(
                tc.tile_pool(name=f"pmt{kb}", bufs=3))
            for kb in range(NT)
        }
        r_pool = ctx.enter_context(tc.tile_pool(name="rinv", bufs=4))
        po_pool = ctx.enter_context(tc.tile_pool(name="posb", bufs=1))

        # ---- load inputs -------------------------------------------------
        xT_sb = const.tile([E + 1, T + QL - 1], BF16)
        nc.sync.dma_start(xT_sb[:], xT[:])
        xT2_sb = const.tile([128, T + QL - 1], BF16)
        nc.sync.dma_start(xT2_sb[:], xT2[:])
        wqk_sb = const.tile([128, QL // 2, 2 * HG * E], BF16)
        nc.sync.dma_start(wqk_sb[:], wqk[:])
        cbias_sb = const.tile([128, 4], F32)
        nc.sync.dma_start(cbias_sb[:], cbias[:])
        wv_sb = const.tile([E + 1, HG * (E + 1)], BF16)
        nc.sync.dma_start(wv_sb[:], wv[:])
        mt_sb = const.tile([128, MT_W], BF16)
        nc.sync.dma_start(mt_sb[:], mt[:])

        # ---- main psum pools (shared with conv/value via tag) -----------
        main_ctx = ExitStack()
        ps_st = main_ctx.enter_context(
            tc.tile_pool(name="psst", bufs=6, space="PSUM"))
        ps_u = main_ctx.enter_context(
            tc.tile_pool(name="psu", bufs=2, space="PSUM"))

        poT_sb = po_pool.tile([E, HG, T], BF16)

        # qkT in two tiles so heads 0/1 (ct 0,2) never depend on the
        # conv writes for heads 2/3 (ct 1,3).
        qkT02 = qk_pool.tile([128, 2, T], BF16, name="qkT02")
        qkT13 = qk_pool.tile([128, 2, T], BF16, name="qkT13")
        v_sb = v_pool.tile([128, NT, HG * (E + 1)], BF16)

        def emit_conv(ct, tt):
            # 6-tap causal conv for one (channel-tile, t-tile): qk channels
            dst = qkT02 if ct in (0, 2) else qkT13
            di = 0 if ct < 2 else 1
            cps = ps_st.tile([128, 512], F32, name="cps", tag="st")
            for p in range(QL // 2):
                nc.tensor.matmul(
                    cps[:, :],
                    lhsT=wqk_sb[:, p, ct * 128:(ct + 1) * 128],
                    rhs=xT2_sb[:, tt * 512 + 2 * p: tt * 512 + 2 * p + 512],
                    start=(p == 0), stop=(p == QL // 2 - 1),
                )
            nc.vector.tensor_scalar_add(
                dst[:, di, tt * 512:(tt + 1) * 512], cps[:, :],
                cbias_sb[:, ct:ct + 1])

        def emit_value(tb):
            vps = ps_st.tile([128, HG * (E + 1)], F32, name="vps", tag="st")
            nc.tensor.matmul(
                vps[:, :],
                lhsT=xT_sb[:, QL - 1 + tb * 128: QL - 1 + (tb + 1) * 128],
                rhs=wv_sb[:],
                start=True, stop=True,
            )
            nc.vector.tensor_copy(v_sb[:, tb, :], vps[:, :])

        pmt_map = [[None] * NT for _ in range(HG)]

        def emit_A2(pair, kb):
            # Paired heads (2*pair, 2*pair+1) live at partition offsets 0/64:
            # their K=64 score matmuls go to distinct PE row-groups and run
            # concurrently.  ST = K^T Q, exp (1/8 folded), log-sparse mask.
            qk_t = qkT02 if pair == 0 else qkT13
            q0 = _qi0(kb)
            wdt = T - q0
            pa = pmt_pools[kb].tile([128, wdt], BF16, name="pmtA", tag="pmt")
            pb = pmt_pools[kb].tile([128, wdt], BF16, name="pmtB", tag="pmt")
            for p0, pt in ((0, pa), (64, pb)):
                for seg in range(q0, T, 512):
                    nn = min(512, T - seg)
                    st_t = ps_st.tile([128, 512], F32, name="st_t", tag="st")
                    nc.tensor.matmul(
                        st_t[:, 0:nn],
                        lhsT=qk_t[p0:p0 + 64, 1, kb * 128:(kb + 1) * 128],
                        rhs=qk_t[p0:p0 + 64, 0, seg: seg + nn],
                        start=True, stop=True)
                    nc.scalar.activation(
                        pt[:, seg - q0: seg - q0 + nn], st_t[:, 0:nn],
                        Exp, scale=0.125)
            nc.vector.tensor_mul(
                pa[:, :], pa[:, :], mt_sb[:, _MT_OFF[kb]: _MT_OFF[kb] + wdt])
            nc.vector.tensor_mul(
                pb[:, :], pb[:, :], mt_sb[:, _MT_OFF[kb]: _MT_OFF[kb] + wdt])
            nc.sync.dma_start(w_out[2 * pair, kb, :, q0:T], pa[:, :])
            nc.sync.dma_start(w_out[2 * pair + 1, kb, :, q0:T], pb[:, :])
            pmt_map[2 * pair][kb] = pa
            pmt_map[2 * pair + 1][kb] = pb

        def emit_B(hh, qt):
            # U = V^T P (unnormalized attn out, transposed); ones-column row
            # gives softmax denominators, transposed into [qi,1] via tiny
            # matmuls against a [1,1] ones tile.
            u_t = ps_u.tile([E + 1, 512], F32)
            for kb in range(4 * qt + 4):
                dd = max(0, kb * 128 - qt * 512)
                src0 = qt * 512 + dd - _qi0(kb)
                nc.tensor.matmul(
                    u_t[:, dd:512],
                    lhsT=v_sb[:, kb, hh * (E + 1):(hh + 1) * (E + 1)],
                    rhs=pmt_map[hh][kb][:, src0: src0 + 512 - dd],
                    start=(kb == 0), stop=(kb == 4 * qt + 3))
            nc.vector.tensor_copy(
                poT_sb[:, hh, qt * 512:(qt + 1) * 512], u_t[0:E, :])
            r_chunk = r_pool.tile([1, 512], F32, tag="rrow")
            nc.vector.tensor_copy(r_chunk[0:1, :], u_t[E:E + 1, :])
            nc.sync.dma_start(
                rv_out[hh, qt * 512:(qt + 1) * 512], r_chunk[0:1, :])

        # Software pipeline: prologue interleaves conv/value with pair-0
        # score tiles; steady state interleaves A2(pair 1) with C(pair 0)
        # descending so pmt pool slots free diagonally.
        prequeue = []
        for tt in range(3, -1, -1):
            prequeue += [("c", 0, tt), ("c", 2, tt), ("v", NT - 1 - (3 - tt), 0)]
        rest = ([("c", 1, tt) for tt in range(4)]
                + [("c", 3, tt) for tt in range(4)]
                + [("v", tb, 0) for tb in range(NT - 4)])
        def emit_item(item):
            if item[0] == "c":
                emit_conv(item[1], item[2])
            else:
                emit_value(item[1])
        for item in prequeue:
            emit_item(item)
        ri = 0
        for kb in range(NT - 1, -1, -1):
            emit_A2(0, kb)
            take = 2 if kb % 2 == 0 else 1
            for _ in range(take):
                if ri < len(rest):
                    emit_item(rest[ri]); ri += 1
        while ri < len(rest):
            emit_item(rest[ri]); ri += 1
        for pair in range(2):
            ha, hb = 2 * pair, 2 * pair + 1
            for qb in range(NT - 1, -1, -1):
                if qb % 4 == 3:
                    emit_B(ha, qb // 4)
                    emit_B(hb, qb // 4)
                if pair == 0:
                    emit_A2(1, qb)
        main_ctx.close()

        nc.sync.dma_start(po_out.rearrange("h e t -> e h t"), poT_sb[:])

    nc.compile()
    return nc


# --------------------------------------------------------------------------
# Host-side sharding / unsharding and the cached PJRT runner.
# --------------------------------------------------------------------------

_RUNNER = None


class _Runner:
    def __init__(self):
        import jax
        from jax.experimental.shard_map import shard_map
        from jax.sharding import Mesh, PartitionSpec
        from concourse.bass2jax import (
            _bass_exec_p, install_neuronx_cc_hook, partition_id_tensor)

        install_neuronx_cc_hook()
        nc = self.nc = _build_nc()
        partition_name = (
            nc.partition_id_tensor.name if nc.partition_id_tensor else None)

        in_names, out_names, out_avals, zero_shapes = [], [], [], []
        for alloc in nc.m.functions[0].allocations:
            if not isinstance(alloc, mybir.MemoryLocationSet):
                continue
            name = alloc.memorylocations[0].name
            if alloc.kind == "ExternalInput":
                if name != partition_name:
                    in_names.append(name)
            elif alloc.kind == "ExternalOutput":
                out_names.append(name)
                shape = tuple(alloc.tensor_shape)
                dtype = mybir.dt.np(alloc.dtype)
                out_avals.append(jax.core.ShapedArray(shape, dtype))
                zero_shapes.append((shape, dtype))
        self.in_names = in_names
        self.out_names = out_names
        self.out_avals = out_avals
        self.zero_shapes = zero_shapes
        n_params = len(in_names)
        n_outs = len(out_names)
        all_names = in_names + out_names
        if partition_name is not None:
            all_names = all_names + [partition_name]

        def _body(*args):
            operands = list(args)
            if partition_name is not None:
                operands.append(partition_id_tensor())
            outs = _bass_exec_p.bind(
                *operands,
                out_avals=tuple(out_avals),
                in_names=tuple(all_names),
                out_names=tuple(out_names),
                lowering_input_output_aliases=(),
                sim_require_finite=True,
                sim_require_nnan=True,
                nc=nc,
            )
            return tuple(outs)

        devices = jax.devices()[:NCORES]
        mesh = Mesh(np.asarray(devices), ("core",))
        in_specs = (PartitionSpec("core"),) * (n_params + n_outs)
        out_specs = (PartitionSpec("core"),) * n_outs
        donate = tuple(range(n_params, n_params + n_outs))
        self.sharded = jax.jit(
            shard_map(_body, mesh=mesh, in_specs=in_specs,
                      out_specs=out_specs, check_rep=False),
            donate_argnums=donate, keep_unused=True,
        )

    def execute(self, in_maps):
        concat_in = [
            np.concatenate([np.asarray(m[name]) for m in in_maps], axis=0)
            for name in self.in_names
        ]
        concat_zeros = [
            np.zeros((NCORES * s[0], *s[1:]), d) for (s, d) in self.zero_shapes
        ]
        out_arrs = self.sharded(*concat_in, *concat_zeros)
        return [
            {
                name: np.asarray(out_arrs[i]).reshape(
                    NCORES, *self.out_avals[i].shape)[c]
                for i, name in enumerate(self.out_names)
            }
            for c in range(NCORES)
        ]


def _get_runner():
    global _RUNNER
    if _RUNNER is None:
        _RUNNER = _Runner()
    return _RUNNER


def _prep_in_maps(x, Wqk, bqk, Wv, bv, Wp, bp, mask):
    x = np.asarray(x, np.float32)
    Wqk = np.asarray(Wqk, np.float32)
    bqk = np.asarray(bqk, np.float32)
    Wv = np.asarray(Wv, np.float32)
    bv = np.asarray(bv, np.float32)
    Wp = np.asarray(Wp, np.float32)
    m = np.asarray(mask).reshape(T, T).astype(np.float32)

    # mask, transposed + causally packed (shared by all cores)
    mt_np = np.zeros((128, MT_W), dtype=BNP)
    for kb in range(NT):
        q0 = _qi0(kb)
        blk = m[q0:, kb * 128:(kb + 1) * 128].T  # [128, T-q0]
        mt_np[:, _MT_OFF[kb]: _MT_OFF[kb] + T - q0] = blk.astype(BNP)
    ident_np = np.eye(128, dtype=BNP)

    in_maps = []
    for core in range(NCORES):
        b, g = divmod(core, 2)
        heads = [4 * g + i for i in range(HG)]

        xT_np = np.zeros((E + 1, T + QL - 1), np.float32)
        xT_np[:E, QL - 1:] = x[b].T
        xT_np[E, :] = 1.0

        # channel order: q of the 4 heads (64 each), then k of the 4 heads
        chan = np.concatenate(
            [np.arange(64 * h, 64 * h + 64) for h in heads]
            + [np.arange(512 + 64 * h, 512 + 64 * h + 64) for h in heads])
        # tap-paired conv weights: rows 0:64 = tap 2p, rows 64:128 = tap 2p+1
        wqk_np = np.zeros((128, QL // 2, 2 * HG * E), np.float32)
        wt = Wqk[chan].transpose(1, 2, 0)  # [e, dt, ci]
        for p in range(QL // 2):
            wqk_np[:E, p] = wt[:, 2 * p]
            wqk_np[E:, p] = wt[:, 2 * p + 1]
        cbias_np = bqk[chan].reshape(4, 128).T.astype(np.float32).copy()
        xT2_np = np.zeros((128, T + QL - 1), np.float32)
        xT2_np[:E] = xT_np[:E]
        xT2_np[E:, :-1] = xT_np[:E, 1:]

        wv_np = np.zeros((E + 1, HG * (E + 1)), np.float32)
        for i, h in enumerate(heads):
            wvp = Wv[:, 64 * h: 64 * h + 64] @ Wp[64 * h: 64 * h + 64, :]
            wv_np[:E, i * (E + 1): i * (E + 1) + E] = wvp
            wv_np[E, i * (E + 1): i * (E + 1) + E] = (
                bv[64 * h: 64 * h + 64] @ Wp[64 * h: 64 * h + 64, :])
            wv_np[E, i * (E + 1) + E] = 1.0

        in_maps.append({
            "xT": xT_np.astype(BNP),
            "wqk": wqk_np.astype(BNP),
            "xT2": xT2_np.astype(BNP),
            "cbias": cbias_np,
            "wv": wv_np.astype(BNP),
            "mt": mt_np,
        })
    return in_maps


def _assemble(results, bp):
    bp = np.asarray(bp, np.float32)
    attn = np.zeros((B, H, T, T), np.float32)
    out = np.empty((B, T, E), np.float32)
    for core in range(NCORES):
        b, g = divmod(core, 2)
        rinv = 1.0 / np.asarray(results[core]["rv"], np.float32)  # [HG, T]
        w = results[core]["w"]                    # bf16 [HG, NT, 128, T]
        for hh in range(HG):
            a = attn[b, 4 * g + hh]
            for kb in range(NT):
                q0 = 128 * kb
                a[q0:, q0:q0 + 128] = w[hh, kb, :, q0:].T
            a *= rinv[hh][:, None]
    for b in range(B):
        acc = np.zeros((T, E), np.float32)
        for core in (2 * b, 2 * b + 1):
            poT = np.asarray(results[core]["po"], np.float32)  # [HG, E, T]
            rinv = 1.0 / np.asarray(results[core]["rv"], np.float32)
            for hh in range(HG):
                acc += (poT[hh] * rinv[hh][None, :]).T
        out[b] = acc + bp
    return out, attn


def kernel(x, Wqk, bqk, Wv, bv, Wp, bp, mask):
    runner = _get_runner()
    in_maps = _prep_in_maps(x, Wqk, bqk, Wv, bv, Wp, bp, mask)
    results = runner.execute(in_maps)
    return _assemble(results, bp)
